# revision 5
# baseline (speedup 1.0000x reference)
"""CVKAN 2-layer kernel for 8x TRN2 NeuronCores (data-parallel over batch).

Contract: kernel(**inputs) takes the FULL unsharded inputs from
reference.setup_inputs() and returns the FULL [8192, 64, 2] float32 output.
Self-contained: hardcodes shapes/sharding; no sibling imports.

v2 structure: GPS-produced product chunks use the earliest basis waves but
are consumed late/interleaved by the PE (PSUM accumulation is commutative),
with a dedicated pool, so the slow GPSIMD never serializes the PE stream.
Silu matmuls open each accumulation (PE work during fill); bias-sum setup
runs in the warmup window; the L0->L1 transition is split per 512-half.
"""

import numpy as np
import ml_dtypes

import concourse.bacc as bacc
import concourse.mybir as mybir
import concourse.tile as tile
from concourse.bass_utils import run_bass_kernel_spmd

NCORES = 8
B = 8192
BL = B // NCORES  # batch rows per core
D0, D1, D2, G = 128, 128, 64, 8
UV = G * G
GRID = np.linspace(-2.0, 2.0, G, dtype=np.float32)
RHO = 1.0
HALF = 512  # psum free-dim tile (one bank of fp32)
NHALF = BL // HALF

f32 = mybir.dt.float32
bf16 = mybir.dt.bfloat16
AF = mybir.ActivationFunctionType
ALU = mybir.AluOpType
BF16NP = ml_dtypes.bfloat16

# ---- schedule knobs ----
GPS0 = 12        # L0 product pairs on GPSIMD (earliest waves)
GPS1 = 9         # L1 product pairs on GPSIMD
STRIDE0 = 4      # consume 1 gps pair after every STRIDE0 dve pairs (L0)
STRIDE1 = 6      # same for L1
SQ_DVE = {7}     # u values whose square step runs as DVE STT (rest: ACT Square)
WARMUP = 24

_CACHE = {}


def _wave_pairs():
    # (u, v) pairs ordered by basis availability: max(u, v) ascending
    out = []
    for m in range(G):
        for u in range(m):
            out.append((u, m))
        for v in range(m + 1):
            out.append((m, v))
    return out


def _consumption(dve, gps, stride):
    # interleave: after every `stride` dve pairs, one gps pair
    out = []
    di = gi = 0
    while di < len(dve) or gi < len(gps):
        for _ in range(stride):
            if di < len(dve):
                out.append(("dve", dve[di])); di += 1
        if gi < len(gps):
            out.append(("gps", gps[gi])); gi += 1
    return out


def _build():
    nc = bacc.Bacc("TRN2", target_bir_lowering=False, debug=False)

    xtr = nc.dram_tensor("xtr", [D0, BL], f32, kind="ExternalInput")
    xti = nc.dram_tensor("xti", [D0, BL], f32, kind="ExternalInput")
    w0r_d = nc.dram_tensor("w0r", [D0, UV * D1], bf16, kind="ExternalInput")
    w0i_d = nc.dram_tensor("w0i", [D0, UV * D1], bf16, kind="ExternalInput")
    w1c_d = nc.dram_tensor("w1c", [D1, UV * 2 * D2], bf16, kind="ExternalInput")
    sw0r_d = nc.dram_tensor("sw0r", [D0, D1], bf16, kind="ExternalInput")
    sw0i_d = nc.dram_tensor("sw0i", [D0, D1], bf16, kind="ExternalInput")
    sw1a_d = nc.dram_tensor("sw1a", [D1, 2 * D2], bf16, kind="ExternalInput")
    sw1b_d = nc.dram_tensor("sw1b", [D1, 2 * D2], bf16, kind="ExternalInput")
    sb0r_d = nc.dram_tensor("sb0r", [D0, D1], f32, kind="ExternalInput")
    sb0i_d = nc.dram_tensor("sb0i", [D0, D1], f32, kind="ExternalInput")
    sb1r_d = nc.dram_tensor("sb1r", [D1, D2], f32, kind="ExternalInput")
    sb1i_d = nc.dram_tensor("sb1i", [D1, D2], f32, kind="ExternalInput")
    y = nc.dram_tensor("y", [2 * D2, BL], f32, kind="ExternalOutput")

    wave = _wave_pairs()

    with tile.TileContext(nc) as tc:
        with (
            tc.tile_pool(name="wpool", bufs=1) as wpool,
            tc.tile_pool(name="xpool", bufs=1) as xpool,
            tc.tile_pool(name="bpool", bufs=1) as bpool,
            tc.tile_pool(name="sqpool", bufs=3) as sqpool,
            tc.tile_pool(name="ppool", bufs=20) as ppool,
            tc.tile_pool(name="gpool", bufs=12) as gpool,
            tc.tile_pool(name="spool", bufs=1) as spool,
            tc.tile_pool(name="cpool", bufs=1) as cpool,
            tc.tile_pool(name="psum", bufs=1, space="PSUM") as pspool,
        ):
            # DMA order: x, small silu weights, grid weights, rest
            xr_sb = xpool.tile_from(xtr.ap(), name="xr_sb")
            xi_sb = xpool.tile_from(xti.ap(), name="xi_sb")
            sw0r = wpool.tile_from(sw0r_d.ap(), name="sw0r_sb")
            sw0i = wpool.tile_from(sw0i_d.ap(), name="sw0i_sb")
            sw1a = wpool.tile_from(sw1a_d.ap(), name="sw1a_sb")
            sw1b = wpool.tile_from(sw1b_d.ap(), name="sw1b_sb")
            w0r = wpool.tile_from(w0r_d.ap(), name="w0r_sb")
            w0i = wpool.tile_from(w0i_d.ap(), name="w0i_sb")
            sb0r = wpool.tile_from(sb0r_d.ap(), name="sb0r_sb")
            sb0i = wpool.tile_from(sb0i_d.ap(), name="sb0i_sb")
            sb1r = wpool.tile_from(sb1r_d.ap(), name="sb1r_sb")
            sb1i = wpool.tile_from(sb1i_d.ap(), name="sb1i_sb")
            w1c = wpool.tile_from(w1c_d.ap(), name="w1c_sb")

            # per-grid-point bias vectors (activation bias must be an AP)
            negg = cpool.tile([128, G], f32)
            negg2 = cpool.tile([128, G], f32)
            for u in range(G):
                g = float(GRID[u])
                nc.gpsimd.memset(negg[:, u : u + 1], -g)
                nc.gpsimd.memset(negg2[:, u : u + 1], -(g * g) / RHO)

            # PE warm-up burst so HAM reaches 8/8 before the real stream
            wtile = cpool.tile([128, HALF], bf16)
            nc.gpsimd.memset(wtile[:], 0.0)
            wps = pspool.tile([128, HALF], f32, tag="warm", name="wps")
            for _ in range(WARMUP):
                nc.tensor.matmul(wps[:], wtile[:, 0:128], wtile[:], start=True, stop=True)

            # silu-weight fixups + bias column sums (all during warmup window)
            sw0i_neg = cpool.tile([D0, D1], bf16)
            nc.vector.tensor_scalar_mul(sw0i_neg[:], sw0i[:], -1.0)
            nc.vector.tensor_scalar_mul(sw1b[:, 0:D2], sw1b[:, 0:D2], -1.0)
            ones = cpool.tile([128, 1], f32)
            nc.gpsimd.memset(ones[:], 1.0)
            bias0r = cpool.tile([128, 1], f32)
            bias0i = cpool.tile([128, 1], f32)
            biascat = cpool.tile([128, 1], f32)
            for sb_sb, dst in (
                (sb0r, bias0r[:]),
                (sb0i, bias0i[:]),
                (sb1r, biascat[0:D2, :]),
                (sb1i, biascat[D2 : 2 * D2, :]),
            ):
                pb = pspool.tile([sb_sb.shape[1], 1], f32, tag="pb", name="pb")
                nc.tensor.matmul(pb[:], sb_sb[:], ones[:], start=True, stop=True)
                nc.vector.tensor_copy(dst, pb[:])

            def basis_u(src_sb, btile, u, comp, cols=None):
                # btile[:, u*BL+cols] = exp(-(src - g_u)^2 / rho)
                g = float(GRID[u])
                sl = slice(u * BL, (u + 1) * BL) if cols is None else slice(
                    u * BL + cols.start, u * BL + cols.stop
                )
                dst = btile[:, sl]
                xin = src_sb[:] if cols is None else src_sb[:, cols]
                w = xin.shape[1]
                tag = "sq" if w == BL else "sqh"
                if u in SQ_DVE:
                    t = sqpool.tile([128, w], f32, tag=tag, name="t")
                    nc.vector.scalar_tensor_tensor(
                        t[:], xin, 2.0 * g, xin, ALU.subtract, ALU.mult
                    )
                    nc.scalar.activation(
                        dst, t[:], AF.Exp, scale=-1.0 / RHO, bias=negg2[:, u : u + 1]
                    )
                else:
                    sq = sqpool.tile([128, w], f32, tag=tag, name="sq")
                    nc.scalar.activation(sq[:], xin, AF.Square, bias=negg[:, u : u + 1])
                    nc.scalar.activation(dst, sq[:], AF.Exp, scale=-1.0 / RHO)

            # ================= layer 0 =================
            br = bpool.tile([D0, G * BL], bf16, tag="br", name="br0")
            bi = bpool.tile([D0, G * BL], bf16, tag="bi", name="bi0")
            sr = spool.tile([D0, BL], bf16)
            si = spool.tile([D0, BL], bf16)

            gps_pairs0 = wave[:GPS0]
            dve_pairs0 = wave[GPS0:]
            ptile0 = {}
            done = -1
            for m in range(G):
                basis_u(xr_sb, br, m, "r")
                basis_u(xi_sb, bi, m, "i")
                if m == 1:
                    # silu inputs right after wave-1 basis on the ACT queue
                    nc.scalar.activation(sr[:], xr_sb[:], AF.Silu)
                    nc.scalar.activation(si[:], xi_sb[:], AF.Silu)
                done = m
                for (u, v) in wave:
                    if max(u, v) != m:
                        continue
                    on_gps = (u, v) in gps_pairs0
                    pool = gpool if on_gps else ppool
                    p = pool.tile([D0, BL], bf16, tag="p", name="p")
                    eng = nc.gpsimd if on_gps else nc.vector
                    eng.tensor_mul(
                        p[:], br[:, u * BL : (u + 1) * BL], bi[:, v * BL : (v + 1) * BL]
                    )
                    ptile0[(u, v)] = p

            rr = [pspool.tile([128, HALF], f32, tag=f"rr{h}", name=f"rr{h}") for h in range(NHALF)]
            ri = [pspool.tile([128, HALF], f32, tag=f"ri{h}", name=f"ri{h}") for h in range(NHALF)]

            cons0 = _consumption(dve_pairs0, gps_pairs0, STRIDE0)
            first = True
            for idx, (kind, (u, v)) in enumerate(cons0):
                p = ptile0[(u, v)]
                uv = u * G + v
                last = idx == len(cons0) - 1
                lr = w0r[:, uv * D1 : (uv + 1) * D1]
                li = w0i[:, uv * D1 : (uv + 1) * D1]
                for h in range(NHALF):
                    rhs = p[:, h * HALF : (h + 1) * HALF]
                    nc.tensor.matmul(rr[h][:], lr, rhs, start=first, stop=last)
                    nc.tensor.matmul(ri[h][:], li, rhs, start=first, stop=last)
                first = False
                if idx == 1:
                    # silu matmuls early in the accumulation (PE fill work)
                    for h in range(NHALF):
                        srh = sr[:, h * HALF : (h + 1) * HALF]
                        sih = si[:, h * HALF : (h + 1) * HALF]
                        nc.tensor.matmul(rr[h][:], sw0r[:], srh, start=False, stop=False)
                        nc.tensor.matmul(rr[h][:], sw0i_neg[:], sih, start=False, stop=False)
                        nc.tensor.matmul(ri[h][:], sw0i[:], srh, start=False, stop=False)
                        nc.tensor.matmul(ri[h][:], sw0r[:], sih, start=False, stop=False)

            # ======== transition: hr/hi extraction + early L1 basis ========
            hr = xpool.tile([D1, BL], f32)
            hi = xpool.tile([D1, BL], f32)
            br1 = bpool.tile([D1, G * BL], bf16, tag="br", name="br1")
            bi1 = bpool.tile([D1, G * BL], bf16, tag="bi", name="bi1")
            sr1 = spool.tile([D1, BL], bf16, tag="sr1")
            si1 = spool.tile([D1, BL], bf16, tag="si1")

            for h in range(NHALF):
                sl = slice(h * HALF, (h + 1) * HALF)
                # hr on ACT, hi on DVE so the two chains run in parallel
                nc.scalar.activation(hr[:, sl], rr[h][:], AF.Identity, bias=bias0r[:])
                nc.vector.tensor_scalar_add(hi[:, sl], ri[h][:], bias0i[:])
                # earliest basis rows per half to shorten the PE bubble
                for u in (0, 1):
                    basis_u(hr, br1, u, "r", cols=sl)
                    basis_u(hi, bi1, u, "i", cols=sl)
            for m in range(2, G):
                basis_u(hr, br1, m, "r")
                basis_u(hi, bi1, m, "i")
                if m == 2:
                    nc.scalar.activation(sr1[:], hr[:], AF.Silu)
                    nc.scalar.activation(si1[:], hi[:], AF.Silu)

            # ================= layer 1 =================
            gps_pairs1 = wave[:GPS1]
            dve_pairs1 = wave[GPS1:]
            ptile1 = {}
            for m in range(G):
                for (u, v) in wave:
                    if max(u, v) != m:
                        continue
                    on_gps = (u, v) in gps_pairs1
                    pool = gpool if on_gps else ppool
                    p1 = pool.tile([D1, BL], bf16, tag="p", name="p1")
                    eng = nc.gpsimd if on_gps else nc.vector
                    eng.tensor_mul(
                        p1[:], br1[:, u * BL : (u + 1) * BL], bi1[:, v * BL : (v + 1) * BL]
                    )
                    ptile1[(u, v)] = p1

            cat = [pspool.tile([128, HALF], f32, tag=f"cat{h}", name=f"cat{h}") for h in range(NHALF)]
            cons1 = _consumption(dve_pairs1, gps_pairs1, STRIDE1)
            first = True
            for idx, (kind, (u, v)) in enumerate(cons1):
                p1 = ptile1[(u, v)]
                uv = u * G + v
                last = idx == len(cons1) - 1
                lc = w1c[:, uv * 2 * D2 : (uv + 1) * 2 * D2]
                for h in range(NHALF):
                    rhs = p1[:, h * HALF : (h + 1) * HALF]
                    nc.tensor.matmul(cat[h][:], lc, rhs, start=first, stop=last)
                first = False
                if idx == 1:
                    for h in range(NHALF):
                        sl = slice(h * HALF, (h + 1) * HALF)
                        nc.tensor.matmul(cat[h][:], sw1a[:], sr1[:, sl], start=False, stop=False)
                        nc.tensor.matmul(cat[h][:], sw1b[:], si1[:, sl], start=False, stop=False)

            oT = xpool.tile([2 * D2, BL], f32)
            for h in range(NHALF):
                sl = slice(h * HALF, (h + 1) * HALF)
                nc.scalar.activation(oT[:, sl], cat[h][:], AF.Identity, bias=biascat[:])
            nc.sync.dma_start(y.ap(), oT[:])

    nc.finalize()
    return nc


def _prep_in_maps(inputs):
    x_real = np.asarray(inputs["x_real"], np.float32)
    x_imag = np.asarray(inputs["x_imag"], np.float32)

    def wb(w):  # [i,o,G,G] -> [i, (u,v,o)] bf16
        w = np.asarray(w, np.float32)
        return np.ascontiguousarray(w.transpose(0, 2, 3, 1)).reshape(w.shape[0], -1).astype(BF16NP)

    w0r = wb(inputs["w0_real"])
    w0i = wb(inputs["w0_imag"])
    w1r = np.asarray(inputs["w1_real"], np.float32).transpose(0, 2, 3, 1)
    w1i = np.asarray(inputs["w1_imag"], np.float32).transpose(0, 2, 3, 1)
    w1c = np.ascontiguousarray(
        np.concatenate([w1r, w1i], axis=-1)
    ).reshape(D1, -1).astype(BF16NP)
    sw0r = np.asarray(inputs["sw0_real"], np.float32).astype(BF16NP)
    sw0i = np.asarray(inputs["sw0_imag"], np.float32).astype(BF16NP)
    sw1r = np.asarray(inputs["sw1_real"], np.float32)
    sw1i = np.asarray(inputs["sw1_imag"], np.float32)
    sw1a = np.ascontiguousarray(np.concatenate([sw1r, sw1i], axis=1)).astype(BF16NP)
    sw1b = np.ascontiguousarray(np.concatenate([sw1i, sw1r], axis=1)).astype(BF16NP)
    shared = {
        "w0r": w0r,
        "w0i": w0i,
        "w1c": w1c,
        "sw0r": sw0r,
        "sw0i": sw0i,
        "sw1a": sw1a,
        "sw1b": sw1b,
        "sb0r": np.ascontiguousarray(np.asarray(inputs["sb0_real"], np.float32)),
        "sb0i": np.ascontiguousarray(np.asarray(inputs["sb0_imag"], np.float32)),
        "sb1r": np.ascontiguousarray(np.asarray(inputs["sb1_real"], np.float32)),
        "sb1i": np.ascontiguousarray(np.asarray(inputs["sb1_imag"], np.float32)),
    }
    in_maps = []
    for c in range(NCORES):
        sl = slice(c * BL, (c + 1) * BL)
        m = dict(shared)
        m["xtr"] = np.ascontiguousarray(x_real[sl].T)
        m["xti"] = np.ascontiguousarray(x_imag[sl].T)
        in_maps.append(m)
    return in_maps


def _run(inputs, trace=False):
    if "nc" not in _CACHE:
        _CACHE["nc"] = _build()
    nc = _CACHE["nc"]
    in_maps = _prep_in_maps(inputs)
    res = run_bass_kernel_spmd(nc, in_maps, core_ids=list(range(NCORES)), trace=trace)
    out = np.empty((B, D2, 2), np.float32)
    for c in range(NCORES):
        yc = res.results[c]["y"]  # [128, BL]; rows 0:64 real, 64:128 imag
        out[c * BL : (c + 1) * BL] = yc.reshape(2, D2, BL).transpose(2, 1, 0)
    return out, res


def kernel(**inputs) -> np.ndarray:
    out, _ = _run(inputs, trace=False)
    return out


if __name__ == "__main__":
    rng = np.random.default_rng(0)
    fake = {
        "x_real": rng.uniform(-1.9, 1.9, (B, D0)).astype(np.float32),
        "x_imag": rng.uniform(-1.9, 1.9, (B, D0)).astype(np.float32),
        "w0_real": rng.standard_normal((D0, D1, G, G)).astype(np.float32),
        "w0_imag": rng.standard_normal((D0, D1, G, G)).astype(np.float32),
        "sw0_real": np.ones((D0, D1), np.float32),
        "sw0_imag": np.zeros((D0, D1), np.float32),
        "sb0_real": np.zeros((D0, D1), np.float32),
        "sb0_imag": np.zeros((D0, D1), np.float32),
        "w1_real": rng.standard_normal((D1, D2, G, G)).astype(np.float32),
        "w1_imag": rng.standard_normal((D1, D2, G, G)).astype(np.float32),
        "sw1_real": np.ones((D1, D2), np.float32),
        "sw1_imag": np.zeros((D1, D2), np.float32),
        "sb1_real": np.zeros((D1, D2), np.float32),
        "sb1_imag": np.zeros((D1, D2), np.float32),
    }
    out = kernel(**fake)
    print("out", out.shape, out.dtype, np.abs(out).mean())


# revision 9
# speedup vs baseline: 1.1959x; 1.1959x over previous
"""CVKAN 2-layer kernel for 8x TRN2 NeuronCores (data-parallel over batch).

Contract: kernel(**inputs) takes the FULL unsharded inputs from
reference.setup_inputs() and returns the FULL [8192, 64, 2] float32 output.
Self-contained: hardcodes shapes/sharding; no sibling imports.

v3: basis rows via exp-recurrence B_u = exp((-(x-g_u)^2 + g_u^2)/rho),
which satisfies B_{u+1} = B_u * E with E = exp(2*dg*x/rho). Two ACT anchor
exps (u=0,4) + one E exp per (comp,layer); remaining rows are bf16 DVE
multiplies at 2x rate. The grid constants exp(-(g_u^2+g_v^2)/rho) are folded
into the weights host-side. Products are wide DVE TTs per u-row; the tail
pairs run on GPSIMD, consumed late/interleaved (PSUM accumulation is
commutative) so the slow GPSIMD never serializes the PE stream.
"""

import numpy as np
import ml_dtypes

import concourse.bacc as bacc
import concourse.mybir as mybir
import concourse.tile as tile
from concourse.bass import AP
from concourse.bass_utils import run_bass_kernel_spmd

NCORES = 8
B = 8192
BL = B // NCORES  # batch rows per core
D0, D1, D2, G = 128, 128, 64, 8
UV = G * G
GRID = np.linspace(-2.0, 2.0, G, dtype=np.float32)
DG = float(GRID[1] - GRID[0])
RHO = 1.0
HALF = 512  # psum free-dim tile (one bank of fp32)
NHALF = BL // HALF

f32 = mybir.dt.float32
bf16 = mybir.dt.bfloat16
AF = mybir.ActivationFunctionType
ALU = mybir.AluOpType
BF16NP = ml_dtypes.bfloat16

# ---- schedule knobs ----
GPS0 = 14        # L0 product pairs on GPSIMD (tail of u-major order)
GPS1 = 10        # L1 product pairs on GPSIMD
STRIDE0 = 4      # consume 1 gps pair after every STRIDE0 dve pairs (L0)
STRIDE1 = 5
ANCHORS = (0, 4)  # basis rows computed directly; rest via recurrence
WARMUP = 24
NARROW_ROWS1 = (0,)  # L1 u-rows produced as narrow chunks (fast first pairs)

_CACHE = {}


def _pairs_umajor():
    return [(u, v) for u in range(G) for v in range(G)]


def _consumption(dve, gps, stride):
    out = []
    di = gi = 0
    while di < len(dve) or gi < len(gps):
        for _ in range(stride):
            if di < len(dve):
                out.append(dve[di]); di += 1
        if gi < len(gps):
            out.append(gps[gi]); gi += 1
    return out


def _build():
    nc = bacc.Bacc("TRN2", target_bir_lowering=False, debug=False)

    xtr = nc.dram_tensor("xtr", [D0, BL], f32, kind="ExternalInput")
    xti = nc.dram_tensor("xti", [D0, BL], f32, kind="ExternalInput")
    w0r_d = nc.dram_tensor("w0r", [D0, UV * D1], bf16, kind="ExternalInput")
    w0i_d = nc.dram_tensor("w0i", [D0, UV * D1], bf16, kind="ExternalInput")
    w1c_d = nc.dram_tensor("w1c", [D1, UV * 2 * D2], bf16, kind="ExternalInput")
    sw0r_d = nc.dram_tensor("sw0r", [D0, D1], bf16, kind="ExternalInput")
    sw0i_d = nc.dram_tensor("sw0i", [D0, D1], bf16, kind="ExternalInput")
    sw1a_d = nc.dram_tensor("sw1a", [D1, 2 * D2], bf16, kind="ExternalInput")
    sw1b_d = nc.dram_tensor("sw1b", [D1, 2 * D2], bf16, kind="ExternalInput")
    sb0r_d = nc.dram_tensor("sb0r", [D0, D1], f32, kind="ExternalInput")
    sb0i_d = nc.dram_tensor("sb0i", [D0, D1], f32, kind="ExternalInput")
    sb1r_d = nc.dram_tensor("sb1r", [D1, D2], f32, kind="ExternalInput")
    sb1i_d = nc.dram_tensor("sb1i", [D1, D2], f32, kind="ExternalInput")
    y = nc.dram_tensor("y", [2 * D2, BL], f32, kind="ExternalOutput")

    pairs = _pairs_umajor()

    with tile.TileContext(nc) as tc:
        with (
            tc.tile_pool(name="wpool", bufs=1) as wpool,
            tc.tile_pool(name="xpool", bufs=1) as xpool,
            tc.tile_pool(name="bpool", bufs=1) as bpool,
            tc.tile_pool(name="sqpool", bufs=2) as sqpool,
            tc.tile_pool(name="ppool", bufs=3) as ppool,      # wide u-row products
            tc.tile_pool(name="npool", bufs=10) as npool,     # narrow products
            tc.tile_pool(name="gpool", bufs=14) as gpool,     # gpsimd products
            tc.tile_pool(name="spool", bufs=1) as spool,
            tc.tile_pool(name="cpool", bufs=1) as cpool,
            tc.tile_pool(name="psum", bufs=1, space="PSUM") as pspool,
        ):
            # DMA order: x, small silu weights, grid weights, bias, w1c
            xr_sb = xpool.tile_from(xtr.ap(), name="xr_sb")
            xi_sb = xpool.tile_from(xti.ap(), name="xi_sb")
            sw0r = wpool.tile_from(sw0r_d.ap(), name="sw0r_sb")
            sw0i = wpool.tile_from(sw0i_d.ap(), name="sw0i_sb")
            sw1a = wpool.tile_from(sw1a_d.ap(), name="sw1a_sb")
            sw1b = wpool.tile_from(sw1b_d.ap(), name="sw1b_sb")
            w0r = wpool.tile_from(w0r_d.ap(), name="w0r_sb")
            w0i = wpool.tile_from(w0i_d.ap(), name="w0i_sb")
            sb0r = wpool.tile_from(sb0r_d.ap(), name="sb0r_sb")
            sb0i = wpool.tile_from(sb0i_d.ap(), name="sb0i_sb")
            sb1r = wpool.tile_from(sb1r_d.ap(), name="sb1r_sb")
            sb1i = wpool.tile_from(sb1i_d.ap(), name="sb1i_sb")
            w1c = wpool.tile_from(w1c_d.ap(), name="w1c_sb")

            # per-anchor bias columns: -g_a (square) and +g_a^2/rho (exp)
            nega = cpool.tile([128, len(ANCHORS)], f32)
            posa2 = cpool.tile([128, len(ANCHORS)], f32)
            for k, a in enumerate(ANCHORS):
                g = float(GRID[a])
                nc.gpsimd.memset(nega[:, k : k + 1], -g)
                nc.gpsimd.memset(posa2[:, k : k + 1], (g * g) / RHO)

            # PE warm-up burst so HAM reaches 8/8 before the real stream
            wtile = cpool.tile([128, HALF], bf16)
            nc.gpsimd.memset(wtile[:], 0.0)
            wps = pspool.tile([128, HALF], f32, tag="warm", name="wps")
            for _ in range(WARMUP):
                nc.tensor.matmul(wps[:], wtile[:, 0:128], wtile[:], start=True, stop=True)

            # silu-weight fixups + bias column sums (all during warmup window)
            sw0i_neg = cpool.tile([D0, D1], bf16)
            nc.vector.tensor_scalar_mul(sw0i_neg[:], sw0i[:], -1.0)
            nc.vector.tensor_scalar_mul(sw1b[:, 0:D2], sw1b[:, 0:D2], -1.0)
            ones = cpool.tile([128, 1], f32)
            nc.gpsimd.memset(ones[:], 1.0)
            bias0r = cpool.tile([128, 1], f32)
            bias0i = cpool.tile([128, 1], f32)
            biascat = cpool.tile([128, 1], f32)
            for sb_sb, dst in (
                (sb0r, bias0r[:]),
                (sb0i, bias0i[:]),
                (sb1r, biascat[0:D2, :]),
                (sb1i, biascat[D2 : 2 * D2, :]),
            ):
                pb = pspool.tile([sb_sb.shape[1], 1], f32, tag="pb", name="pb")
                nc.tensor.matmul(pb[:], sb_sb[:], ones[:], start=True, stop=True)
                nc.vector.tensor_copy(dst, pb[:])

            def make_basis(src_sb, btile, etile, eclamp=None):
                # btile row u = exp((-(src-g_u)^2 + g_u^2)/rho), via anchors
                # u=0,4 on ACT and bf16 DVE recurrence B_{u+1} = B_u * E.
                # eclamp caps the E input: exact, because anchors underflow to
                # exactly 0 wherever |src| is large enough for the cap to bind.
                for k, a in enumerate(ANCHORS):
                    sq = sqpool.tile([128, BL], f32, tag="sq", name="sq")
                    nc.scalar.activation(sq[:], src_sb[:], AF.Square, bias=nega[:, k : k + 1])
                    nc.scalar.activation(
                        btile[:, a * BL : (a + 1) * BL], sq[:], AF.Exp,
                        scale=-1.0 / RHO, bias=posa2[:, k : k + 1],
                    )
                esrc = src_sb
                if eclamp is not None:
                    ec = sqpool.tile([128, BL], f32, tag="ec", name="ec")
                    nc.vector.tensor_single_scalar(ec[:], src_sb[:], eclamp, ALU.min)
                    esrc = ec
                nc.scalar.activation(etile[:], esrc[:], AF.Exp, scale=2.0 * DG / RHO)
                for k, a in enumerate(ANCHORS):
                    hi_u = (ANCHORS[k + 1] if k + 1 < len(ANCHORS) else G)
                    for u in range(a + 1, hi_u):
                        nc.vector.tensor_mul(
                            btile[:, u * BL : (u + 1) * BL],
                            btile[:, (u - 1) * BL : u * BL],
                            etile[:],
                        )

            def wide_product(p, btile, bitile, u, v0, nv):
                # p[:, 0:nv*BL] = B_u (bcast) * Bi rows v0..v0+nv-1
                src0 = AP(btile.tensor, btile[:, u * BL : (u + 1) * BL].offset,
                          [btile[:].ap[0], [0, nv], [1, BL]])
                src1 = AP(bitile.tensor, bitile[:, v0 * BL : (v0 + 1) * BL].offset,
                          [bitile[:].ap[0], [BL, nv], [1, BL]])
                dst = AP(p.tensor, p[:].offset, [p[:].ap[0], [BL, nv], [1, BL]])
                nc.vector.tensor_tensor(dst, src0, src1, ALU.mult)

            def layer_products(btile, bitile, gps_n, narrow_rows):
                # returns rhs lookup: (u, v) -> (tile, col offset)
                gps_pairs = pairs[UV - gps_n:]
                rhs = {}
                for u in range(G):
                    row = [(u, v) for v in range(G) if (u, v) not in gps_pairs]
                    if not row:
                        continue
                    if u in narrow_rows:
                        for (uu, v) in row:
                            p = npool.tile([128, BL], bf16, tag="np", name="np")
                            nc.vector.tensor_mul(
                                p[:], btile[:, u * BL : (u + 1) * BL],
                                bitile[:, v * BL : (v + 1) * BL])
                            rhs[(u, v)] = (p, 0)
                    else:
                        # v-groups of <=4 rows per wide TT (SBUF-friendly)
                        for g0 in range(0, len(row), 4):
                            grp = row[g0 : g0 + 4]
                            v0, nv = grp[0][1], len(grp)
                            p = ppool.tile([128, 4 * BL], bf16, tag="wp", name="wp")
                            wide_product(p, btile, bitile, u, v0, nv)
                            for j, (uu, v) in enumerate(grp):
                                rhs[(u, v)] = (p, j * BL)
                for (u, v) in gps_pairs:
                    p = gpool.tile([128, BL], bf16, tag="gp", name="gp")
                    nc.gpsimd.tensor_mul(
                        p[:], btile[:, u * BL : (u + 1) * BL],
                        bitile[:, v * BL : (v + 1) * BL])
                    rhs[(u, v)] = (p, 0)
                dve_pairs = [pr for pr in pairs if pr not in gps_pairs]
                return rhs, dve_pairs, gps_pairs

            # ================= layer 0 =================
            br = bpool.tile([D0, G * BL], bf16, tag="br", name="br0")
            bi = bpool.tile([D0, G * BL], bf16, tag="bi", name="bi0")
            er = bpool.tile([D0, BL], bf16, tag="er", name="er0")
            ei = bpool.tile([D0, BL], bf16, tag="ei", name="ei0")
            sr = spool.tile([D0, BL], bf16, tag="sr")
            si = spool.tile([D0, BL], bf16, tag="si")

            make_basis(xr_sb, br, er)
            nc.scalar.activation(sr[:], xr_sb[:], AF.Silu)
            nc.scalar.activation(si[:], xi_sb[:], AF.Silu)
            make_basis(xi_sb, bi, ei)

            rhs0, dve0, gps0 = layer_products(br, bi, GPS0, ())

            rr = [pspool.tile([128, HALF], f32, tag=f"rr{h}", name=f"rr{h}") for h in range(NHALF)]
            ri = [pspool.tile([128, HALF], f32, tag=f"ri{h}", name=f"ri{h}") for h in range(NHALF)]

            cons0 = _consumption(dve0, gps0, STRIDE0)
            first = True
            for idx, (u, v) in enumerate(cons0):
                p, off = rhs0[(u, v)]
                uv = u * G + v
                last = idx == len(cons0) - 1
                lr = w0r[:, uv * D1 : (uv + 1) * D1]
                li = w0i[:, uv * D1 : (uv + 1) * D1]
                for h in range(NHALF):
                    r_ap = p[:, off + h * HALF : off + (h + 1) * HALF]
                    nc.tensor.matmul(rr[h][:], lr, r_ap, start=first, stop=last)
                    nc.tensor.matmul(ri[h][:], li, r_ap, start=first, stop=last)
                first = False
                if idx == 1:
                    for h in range(NHALF):
                        srh = sr[:, h * HALF : (h + 1) * HALF]
                        sih = si[:, h * HALF : (h + 1) * HALF]
                        nc.tensor.matmul(rr[h][:], sw0r[:], srh, start=False, stop=False)
                        nc.tensor.matmul(rr[h][:], sw0i_neg[:], sih, start=False, stop=False)
                        nc.tensor.matmul(ri[h][:], sw0i[:], srh, start=False, stop=False)
                        nc.tensor.matmul(ri[h][:], sw0r[:], sih, start=False, stop=False)

            # ======== transition: hr/hi extraction + L1 basis ========
            hr = xpool.tile([D1, BL], f32)
            hi = xpool.tile([D1, BL], f32)
            br1 = bpool.tile([D1, G * BL], bf16, tag="br", name="br1")
            bi1 = bpool.tile([D1, G * BL], bf16, tag="bi", name="bi1")
            er1 = bpool.tile([D1, BL], bf16, tag="er", name="er1")
            ei1 = bpool.tile([D1, BL], bf16, tag="ei", name="ei1")
            sr1 = spool.tile([D1, BL], bf16, tag="sr1")
            si1 = spool.tile([D1, BL], bf16, tag="si1")

            for h in range(NHALF):
                sl = slice(h * HALF, (h + 1) * HALF)
                nc.scalar.activation(hr[:, sl], rr[h][:], AF.Identity, bias=bias0r[:])
                nc.vector.tensor_scalar_add(hi[:, sl], ri[h][:], bias0i[:])
            make_basis(hr, br1, er1, eclamp=20.0)
            make_basis(hi, bi1, ei1, eclamp=20.0)
            nc.scalar.activation(sr1[:], hr[:], AF.Silu)
            nc.scalar.activation(si1[:], hi[:], AF.Silu)

            # ================= layer 1 =================
            rhs1, dve1, gps1 = layer_products(br1, bi1, GPS1, NARROW_ROWS1)

            cat = [pspool.tile([128, HALF], f32, tag=f"cat{h}", name=f"cat{h}") for h in range(NHALF)]
            cons1 = _consumption(dve1, gps1, STRIDE1)
            first = True
            for idx, (u, v) in enumerate(cons1):
                p1, off = rhs1[(u, v)]
                uv = u * G + v
                last = idx == len(cons1) - 1
                lc = w1c[:, uv * 2 * D2 : (uv + 1) * 2 * D2]
                for h in range(NHALF):
                    r_ap = p1[:, off + h * HALF : off + (h + 1) * HALF]
                    nc.tensor.matmul(cat[h][:], lc, r_ap, start=first, stop=last)
                first = False
                if idx == 1:
                    for h in range(NHALF):
                        sl = slice(h * HALF, (h + 1) * HALF)
                        nc.tensor.matmul(cat[h][:], sw1a[:], sr1[:, sl], start=False, stop=False)
                        nc.tensor.matmul(cat[h][:], sw1b[:], si1[:, sl], start=False, stop=False)

            oT = xpool.tile([2 * D2, BL], f32)
            for h in range(NHALF):
                sl = slice(h * HALF, (h + 1) * HALF)
                nc.scalar.activation(oT[:, sl], cat[h][:], AF.Identity, bias=biascat[:])
            nc.sync.dma_start(y.ap(), oT[:])

    nc.finalize()
    return nc


def _prep_in_maps(inputs):
    x_real = np.asarray(inputs["x_real"], np.float32)
    x_imag = np.asarray(inputs["x_imag"], np.float32)

    # fold the grid constants exp(-(g_u^2+g_v^2)/rho) into the weights
    gsc = np.exp(-(GRID.astype(np.float64) ** 2) / RHO)
    guv = (gsc[:, None] * gsc[None, :]).astype(np.float32)  # [G, G]

    def wb(w):  # [i,o,G,G] -> [i, (u,v,o)] bf16, grid-scaled
        w = np.asarray(w, np.float32) * guv[None, None, :, :]
        return np.ascontiguousarray(w.transpose(0, 2, 3, 1)).reshape(w.shape[0], -1).astype(BF16NP)

    w0r = wb(inputs["w0_real"])
    w0i = wb(inputs["w0_imag"])
    w1r = (np.asarray(inputs["w1_real"], np.float32) * guv[None, None, :, :]).transpose(0, 2, 3, 1)
    w1i = (np.asarray(inputs["w1_imag"], np.float32) * guv[None, None, :, :]).transpose(0, 2, 3, 1)
    w1c = np.ascontiguousarray(
        np.concatenate([w1r, w1i], axis=-1)
    ).reshape(D1, -1).astype(BF16NP)
    sw0r = np.asarray(inputs["sw0_real"], np.float32).astype(BF16NP)
    sw0i = np.asarray(inputs["sw0_imag"], np.float32).astype(BF16NP)
    sw1r = np.asarray(inputs["sw1_real"], np.float32)
    sw1i = np.asarray(inputs["sw1_imag"], np.float32)
    sw1a = np.ascontiguousarray(np.concatenate([sw1r, sw1i], axis=1)).astype(BF16NP)
    sw1b = np.ascontiguousarray(np.concatenate([sw1i, sw1r], axis=1)).astype(BF16NP)
    shared = {
        "w0r": w0r,
        "w0i": w0i,
        "w1c": w1c,
        "sw0r": sw0r,
        "sw0i": sw0i,
        "sw1a": sw1a,
        "sw1b": sw1b,
        "sb0r": np.ascontiguousarray(np.asarray(inputs["sb0_real"], np.float32)),
        "sb0i": np.ascontiguousarray(np.asarray(inputs["sb0_imag"], np.float32)),
        "sb1r": np.ascontiguousarray(np.asarray(inputs["sb1_real"], np.float32)),
        "sb1i": np.ascontiguousarray(np.asarray(inputs["sb1_imag"], np.float32)),
    }
    in_maps = []
    for c in range(NCORES):
        sl = slice(c * BL, (c + 1) * BL)
        m = dict(shared)
        m["xtr"] = np.ascontiguousarray(x_real[sl].T)
        m["xti"] = np.ascontiguousarray(x_imag[sl].T)
        in_maps.append(m)
    return in_maps


def _run(inputs, trace=False):
    if "nc" not in _CACHE:
        _CACHE["nc"] = _build()
    nc = _CACHE["nc"]
    in_maps = _prep_in_maps(inputs)
    res = run_bass_kernel_spmd(nc, in_maps, core_ids=list(range(NCORES)), trace=trace)
    out = np.empty((B, D2, 2), np.float32)
    for c in range(NCORES):
        yc = res.results[c]["y"]  # [128, BL]; rows 0:64 real, 64:128 imag
        out[c * BL : (c + 1) * BL] = yc.reshape(2, D2, BL).transpose(2, 1, 0)
    return out, res


def kernel(**inputs) -> np.ndarray:
    out, _ = _run(inputs, trace=False)
    return out


if __name__ == "__main__":
    rng = np.random.default_rng(0)
    fake = {
        "x_real": rng.uniform(-1.9, 1.9, (B, D0)).astype(np.float32),
        "x_imag": rng.uniform(-1.9, 1.9, (B, D0)).astype(np.float32),
        "w0_real": rng.standard_normal((D0, D1, G, G)).astype(np.float32),
        "w0_imag": rng.standard_normal((D0, D1, G, G)).astype(np.float32),
        "sw0_real": np.ones((D0, D1), np.float32),
        "sw0_imag": np.zeros((D0, D1), np.float32),
        "sb0_real": np.zeros((D0, D1), np.float32),
        "sb0_imag": np.zeros((D0, D1), np.float32),
        "w1_real": rng.standard_normal((D1, D2, G, G)).astype(np.float32),
        "w1_imag": rng.standard_normal((D1, D2, G, G)).astype(np.float32),
        "sw1_real": np.ones((D1, D2), np.float32),
        "sw1_imag": np.zeros((D1, D2), np.float32),
        "sb1_real": np.zeros((D1, D2), np.float32),
        "sb1_imag": np.zeros((D1, D2), np.float32),
    }
    out = kernel(**fake)
    print("out", out.shape, out.dtype, np.abs(out).mean())
